# revision 2
# baseline (speedup 1.0000x reference)
"""DMPNN conv kernel for TRN2, 8 NeuronCores, edge-parallel with dst-bucketed
sharding.

Algorithm (matching reference):
  em        = relu(x[src] @ W1.T + edge_attr @ W2.T)       [E, 64]
  agg       = segment_sum(em, dst)                          [N, 64]
  m2        = relu((agg @ Wh.T)[src]) + em                  [E, 64]
  nm        = LN(m2) * gamma + beta                         [E, 64]  (output 2)
  nodemsg   = segment_sum(nm, dst)                          [N, 64]
  node_out  = relu([x, nodemsg] @ Wo.T + b_o)               [N, 64]  (output 1)

Key trick: (agg @ Wh.T) is computed per-node BEFORE the per-edge gather, so
both per-edge "matmuls" involving gathers become plain row gathers
(dma_gather); segment sums use dma_scatter_add.

Sharding: nodes remapped to per-core ranges of N_PC rows (real nodes packed at
the start of each range); edges bucketed by dst core; gather tables split at
row 32768 into lo/hi so indices fit int16.

HW constraint (measured): dma_scatter_add races when the SAME call contains
duplicate indices, but separate calls serialize correctly (Tile WAW deps).
So the host assigns edges to scatter calls of SBLK slots such that no call
repeats a dst; pad slots point at a per-slot-unique trash row.
"""

import numpy as np
import concourse.bass as bass
import concourse.bacc as bacc
import concourse.tile as tile
import concourse.mybir as mybir
from concourse import bass_utils, library_config
from concourse.masks import make_identity

F32 = mybir.dt.float32
I16 = mybir.dt.int16
AF = mybir.ActivationFunctionType
ALU = mybir.AluOpType


def build_graph(cfg):
    """Build the SPMD Bacc graph. cfg keys:
    N_TAB: padded global node-table rows (e.g. 65536), multiple of 2*N_LO
    N_LO:  lo-table rows (e.g. 32768)
    N_PC:  padded node rows per core (N_TAB // 8)
    N_AGGROWS: agg/nodemsg table rows per core (N_PC + trash pad, mult of 128)
    P_L, P_H: padded L/H edge-section sizes per core (multiples of GBLK)
    GBLK: edges per gather/scatter call (multiple of 1024)
    D: hidden (64), EDIM: edge attr dim (16)
    LN_EPS: epsilon
    use_gamma_beta, use_bo: emit those ops
    ncores
    """
    D = cfg["D"]
    EDIM = cfg["EDIM"]
    GBLK = cfg["GBLK"]
    P_L, P_H = cfg["P_L"], cfg["P_H"]
    P_E = P_L + P_H
    N_TAB = cfg["N_TAB"]
    N_LO = cfg["N_LO"]
    N_PC = cfg["N_PC"]
    N_AGGROWS = cfg["N_AGGROWS"]
    NCORES = cfg["ncores"]
    NB_L, NB_H = P_L // GBLK, P_H // GBLK
    NB = NB_L + NB_H
    CB = GBLK // 128          # gather-layout chunks per block
    IC = GBLK // 16           # idx cols per block
    assert GBLK % 1024 == 0 and D == 64
    assert cfg["SBLK"] <= GBLK and GBLK % cfg["SBLK"] == 0

    nc = bacc.Bacc("TRN2", target_bir_lowering=False, debug=False,
                   num_devices=NCORES)

    # ---------------- I/O ----------------
    xT = nc.dram_tensor("xT", [D, N_TAB], F32, kind="ExternalInput").ap()
    xTc = nc.dram_tensor("xTc", [D, N_PC], F32, kind="ExternalInput").ap()
    attrT = nc.dram_tensor("attrT", [EDIM, P_E], F32, kind="ExternalInput").ap()
    gidx_lo = nc.dram_tensor("gidx_lo", [128, P_L // 16], I16, kind="ExternalInput").ap()
    gidx_hi = nc.dram_tensor("gidx_hi", [128, P_H // 16], I16, kind="ExternalInput").ap()
    sidx = nc.dram_tensor("sidx", [128, P_E // 16], I16, kind="ExternalInput").ap()
    w1t = nc.dram_tensor("w1t", [D, D], F32, kind="ExternalInput").ap()
    w2t = nc.dram_tensor("w2t", [EDIM, D], F32, kind="ExternalInput").ap()
    wht = nc.dram_tensor("wht", [D, D], F32, kind="ExternalInput").ap()
    wot = nc.dram_tensor("wot", [2 * D, D], F32, kind="ExternalInput").ap()
    gb = nc.dram_tensor("gb", [128, 2 * D], F32, kind="ExternalInput").ap()  # gamma/beta replicated
    boc = nc.dram_tensor("boc", [D, 1], F32, kind="ExternalInput").ap()

    nm_out = nc.dram_tensor("nm_out", [128, P_E // 128, D], F32, kind="ExternalOutput").ap()
    node_outT = nc.dram_tensor("node_outT", [D, N_PC], F32, kind="ExternalOutput").ap()

    # ---------------- internal DRAM ----------------
    xw1 = nc.dram_tensor("xw1", [N_TAB, D], F32).ap()
    em_buf = nc.dram_tensor("em_buf", [128, P_E // 128, D], F32).ap()
    agg = nc.dram_tensor("agg", [N_AGGROWS, D], F32).ap()
    nodemsg = nc.dram_tensor("nodemsg", [N_AGGROWS, D], F32).ap()
    aggw_slice = nc.dram_tensor("aggw_slice", [N_PC, D], F32).ap()
    aggw_full = nc.dram_tensor("aggw_full", [N_TAB, D], F32, addr_space="Shared").ap()

    with tile.TileContext(nc) as tc:
        _body(tc, cfg, locals())
    nc.compile()
    return nc


def _body(tc, cfg, t):
    nc = tc.nc
    D = cfg["D"]; EDIM = cfg["EDIM"]; GBLK = cfg["GBLK"]
    P_L, P_H = cfg["P_L"], cfg["P_H"]; P_E = P_L + P_H
    N_TAB = cfg["N_TAB"]; N_LO = cfg["N_LO"]; N_PC = cfg["N_PC"]
    N_AGGROWS = cfg["N_AGGROWS"]; NCORES = cfg["ncores"]
    NB_L, NB_H = P_L // GBLK, P_H // GBLK
    CB = GBLK // 128; IC = GBLK // 16
    LN_EPS = cfg["LN_EPS"]

    SBLK = cfg["SBLK"]
    SPB = cfg["GBLK"] // SBLK
    xT, xTc, attrT = t["xT"], t["xTc"], t["attrT"]
    gidx_lo, gidx_hi, sidx = t["gidx_lo"], t["gidx_hi"], t["sidx"]
    w1t, w2t, wht, wot, gb, boc = t["w1t"], t["w2t"], t["wht"], t["wot"], t["gb"], t["boc"]
    nm_out, node_outT = t["nm_out"], t["node_outT"]
    xw1, em_buf, agg, nodemsg = t["xw1"], t["em_buf"], t["agg"], t["nodemsg"]
    aggw_slice, aggw_full = t["aggw_slice"], t["aggw_full"]

    import contextlib
    ctx = contextlib.ExitStack()
    with ctx:
        consts = ctx.enter_context(tc.tile_pool(name="consts", bufs=1))
        sb = ctx.enter_context(tc.tile_pool(name="sb", bufs=3))
        gpool = ctx.enter_context(tc.tile_pool(name="gpool", bufs=2))
        empool = ctx.enter_context(tc.tile_pool(name="empool", bufs=2))
        psum = ctx.enter_context(tc.tile_pool(name="psum", bufs=4, space="PSUM"))
        psum_t = ctx.enter_context(tc.tile_pool(name="psum_t", bufs=2, space="PSUM"))
        small = ctx.enter_context(tc.tile_pool(name="small", bufs=4))

        nc.gpsimd.load_library(library_config.mlp)

        # consts
        w1t_sb = consts.tile([D, D], F32)
        nc.sync.dma_start(out=w1t_sb[:], in_=w1t[:])
        w2t_sb = consts.tile([EDIM, D], F32)
        nc.sync.dma_start(out=w2t_sb[:], in_=w2t[:])
        wht_sb = consts.tile([D, D], F32)
        nc.sync.dma_start(out=wht_sb[:], in_=wht[:])
        wot_sb = consts.tile([2 * D, D], F32)
        nc.sync.dma_start(out=wot_sb[:], in_=wot[:])
        ident = consts.tile([128, 128], F32)
        make_identity(nc, ident[:])
        eps_sb = consts.tile([128, 1], F32)
        nc.vector.memset(eps_sb[:], LN_EPS)
        gb_sb = consts.tile([128, 2 * D], F32)
        nc.sync.dma_start(out=gb_sb[:], in_=gb[:])
        bo_sb = consts.tile([D, 1], F32)
        nc.sync.dma_start(out=bo_sb[:], in_=boc[:])

        # zero agg + nodemsg tables
        zrows = N_AGGROWS // 128
        zt = consts.tile([128, zrows * D], F32)
        nc.vector.memset(zt[:], 0.0)
        agg_pview = agg.rearrange("(p r) d -> p (r d)", p=128)
        nm_pview = nodemsg.rearrange("(p r) d -> p (r d)", p=128)
        nc.sync.dma_start(out=agg_pview, in_=zt[:])
        nc.sync.dma_start(out=nm_pview, in_=zt[:])

        # ---------------- stage A: xw1 = x @ W1.T (all N_TAB rows) ----------------
        # per 1024-node group: load xT block, 8 matmuls into one PSUM bank,
        # copy to SBUF, DMA out to node-major rows.
        NG = N_TAB // 1024
        for g in range(NG):
            xt_blk = sb.tile([D, 1024], F32, tag="xt_blk")
            nc.sync.dma_start(out=xt_blk[:], in_=xT[:, g * 1024:(g + 1) * 1024])
            ps = psum.tile([128, 8, D], F32, tag="ps")
            for j in range(8):
                nc.tensor.matmul(
                    out=ps[:, j, :],
                    lhsT=xt_blk[:, j * 128:(j + 1) * 128],
                    rhs=w1t_sb[:],
                    start=True, stop=True)
            stg = sb.tile([128, 8, D], F32, tag="stg_a")
            nc.any.tensor_copy(out=stg[:], in_=ps[:])
            # node n = g*1024 + j*128 + p  ->  xw1 rows
            dview = xw1[g * 1024:(g + 1) * 1024, :].rearrange(
                "(j p) d -> p j d", p=128)
            nc.sync.dma_start(out=dview, in_=stg[:])

        # ---------------- pass 1 ----------------
        # blocks 0..NB_L-1 from lo table, NB_L..NB-1 from hi table
        for b in range(NB_L + NB_H):
            is_lo = b < NB_L
            tab = xw1[0:N_LO, :] if is_lo else xw1[N_LO:N_TAB, :]
            gsrc = gidx_lo if is_lo else gidx_hi
            goff = b * IC if is_lo else (b - NB_L) * IC

            gi = small.tile([128, IC], I16, tag="gi1")
            nc.sync.dma_start(out=gi[:], in_=gsrc[:, goff:goff + IC])
            gt = gpool.tile([128, CB, D], F32, tag="gt")
            for h in range(GBLK // 1024):
                nc.gpsimd.dma_gather(gt[:, h * 8:(h + 1) * 8, :], tab,
                                     gi[:, h * 64:(h + 1) * 64], 1024, 1024, D)

            at_blk = sb.tile([EDIM, GBLK], F32, tag="at_blk")
            nc.sync.dma_start(out=at_blk[:], in_=attrT[:, b * GBLK:(b + 1) * GBLK])

            em_stage = empool.tile([128, CB, D], F32, tag="em")
            for q in range(GBLK // 1024):
                ps = psum.tile([128, 8, D], F32, tag="ps")
                for j in range(8):
                    e0 = q * 1024 + j * 128
                    nc.tensor.matmul(
                        out=ps[:, j, :],
                        lhsT=at_blk[:, e0:e0 + 128],
                        rhs=w2t_sb[:],
                        start=True, stop=True)
                c0 = q * 8
                nc.vector.tensor_add(
                    out=em_stage[:, c0:c0 + 8, :],
                    in0=ps[:],
                    in1=gt[:, c0:c0 + 8, :])
                nc.scalar.activation(
                    out=em_stage[:, c0:c0 + 8, :],
                    in_=em_stage[:, c0:c0 + 8, :],
                    func=AF.Relu)
            nc.sync.dma_start(out=em_buf[:, b * CB:(b + 1) * CB, :], in_=em_stage[:])
            si = small.tile([128, IC], I16, tag="si1")
            nc.sync.dma_start(out=si[:], in_=sidx[:, b * IC:(b + 1) * IC])
            for k in range(SPB):
                nc.gpsimd.dma_scatter_add(
                    agg[:],
                    em_stage[:, k * (SBLK // 128):(k + 1) * (SBLK // 128), :],
                    si[:, k * (SBLK // 16):(k + 1) * (SBLK // 16)],
                    SBLK, SBLK, D)

        # ---------------- stage B: aggw = agg @ Wh.T, AllGather ----------------
        NT = N_PC // 128
        for tt in range(NT):
            ag_sb = sb.tile([128, D], F32, tag="ag_sb")
            nc.sync.dma_start(out=ag_sb[:], in_=agg[tt * 128:(tt + 1) * 128, :])
            pt = psum_t.tile([D, 128], F32, tag="pt")
            nc.tensor.transpose(out=pt[:], in_=ag_sb[:], identity=ident[:])
            agT = sb.tile([D, 128], F32, tag="agT")
            nc.any.tensor_copy(out=agT[:], in_=pt[:])
            po = psum_t.tile([128, D], F32, tag="po")
            nc.tensor.matmul(out=po[:], lhsT=agT[:], rhs=wht_sb[:], start=True, stop=True)
            aw = sb.tile([128, D], F32, tag="aw")
            nc.any.tensor_copy(out=aw[:], in_=po[:])
            nc.sync.dma_start(out=aggw_slice[tt * 128:(tt + 1) * 128, :], in_=aw[:])

        nc.gpsimd.collective_compute(
            "AllGather", ALU.bypass,
            replica_groups=[list(range(NCORES))],
            ins=[aggw_slice[:].opt()],
            outs=[aggw_full[:].opt()])

        # ---------------- pass 2 ----------------
        for b in range(NB_L + NB_H):
            is_lo = b < NB_L
            tab = aggw_full[0:N_LO, :] if is_lo else aggw_full[N_LO:N_TAB, :]
            gsrc = gidx_lo if is_lo else gidx_hi
            goff = b * IC if is_lo else (b - NB_L) * IC

            gi = small.tile([128, IC], I16, tag="gi2")
            nc.sync.dma_start(out=gi[:], in_=gsrc[:, goff:goff + IC])
            gt = gpool.tile([128, CB, D], F32, tag="gt")
            for h in range(GBLK // 1024):
                nc.gpsimd.dma_gather(gt[:, h * 8:(h + 1) * 8, :], tab,
                                     gi[:, h * 64:(h + 1) * 64], 1024, 1024, D)

            em_t = empool.tile([128, CB, D], F32, tag="em")
            nc.sync.dma_start(out=em_t[:], in_=em_buf[:, b * CB:(b + 1) * CB, :])

            nm_stage = empool.tile([128, CB, D], F32, tag="nm2")
            # m2 = relu(gt) + em
            nc.scalar.activation(out=gt[:], in_=gt[:], func=AF.Relu)
            nc.vector.tensor_add(out=nm_stage[:], in0=gt[:], in1=em_t[:])

            # layer norm per edge-row over inner D (explicit reduce path;
            # bn_stats can't batch over the chunk dim — AP flattening)
            for s in range(CB // 8):
                sl = nm_stage[:, s * 8:(s + 1) * 8, :]
                s1 = small.tile([128, 8, 1], F32, tag="s1")
                nc.vector.reduce_sum(out=s1[:], in_=sl, axis=mybir.AxisListType.X)
                sq = small.tile([128, 8, D], F32, tag="sq")
                nc.scalar.activation(out=sq[:], in_=sl, func=AF.Square)
                s2 = small.tile([128, 8, 1], F32, tag="s2")
                nc.vector.reduce_sum(out=s2[:], in_=sq[:], axis=mybir.AxisListType.X)
                mean = small.tile([128, 8, 1], F32, tag="mean")
                nc.scalar.activation(out=mean[:], in_=s1[:], func=AF.Copy,
                                     scale=1.0 / D)
                var = small.tile([128, 8, 1], F32, tag="var")
                nc.vector.tensor_mul(out=var[:], in0=mean[:], in1=mean[:])
                # var = s2/D - mean^2  -> ACT: s2*(1/D) + (-1)*msq via two steps
                nc.vector.scalar_tensor_tensor(
                    out=var[:], in0=s2[:], scalar=1.0 / D, in1=var[:],
                    op0=ALU.mult, op1=ALU.subtract)
                rstd = small.tile([128, 8, 1], F32, tag="rstd")
                nc.scalar.activation(out=rstd[:], in_=var[:],
                                     func=AF.Sqrt, bias=eps_sb[:], scale=1.0)
                nc.vector.reciprocal(out=rstd[:], in_=rstd[:])
                nc.vector.tensor_tensor(
                    out=sl, in0=sl,
                    in1=mean[:].to_broadcast([128, 8, D]),
                    op=ALU.subtract)
                nc.vector.tensor_tensor(
                    out=sl, in0=sl,
                    in1=rstd[:].to_broadcast([128, 8, D]),
                    op=ALU.mult)
                if cfg["use_gamma_beta"]:
                    for c in range(8):
                        nc.vector.tensor_mul(
                            out=sl[:, c, :], in0=sl[:, c, :], in1=gb_sb[:, 0:D])
                        nc.vector.tensor_add(
                            out=sl[:, c, :], in0=sl[:, c, :], in1=gb_sb[:, D:2 * D])
            nc.sync.dma_start(out=nm_out[:, b * CB:(b + 1) * CB, :], in_=nm_stage[:])
            si = small.tile([128, IC], I16, tag="si2")
            nc.sync.dma_start(out=si[:], in_=sidx[:, b * IC:(b + 1) * IC])
            for k in range(SPB):
                nc.gpsimd.dma_scatter_add(
                    nodemsg[:],
                    nm_stage[:, k * (SBLK // 128):(k + 1) * (SBLK // 128), :],
                    si[:, k * (SBLK // 16):(k + 1) * (SBLK // 16)],
                    SBLK, SBLK, D)

        # ---------------- stage D: node_out ----------------
        for g in range(N_PC // 1024):
            ostg = sb.tile([D, 1024], F32, tag="ostg")
            for j in range(8):
                tt = g * 8 + j
                nm_sb = sb.tile([128, D], F32, tag="nm_sb")
                nc.sync.dma_start(out=nm_sb[:], in_=nodemsg[tt * 128:(tt + 1) * 128, :])
                pt = psum_t.tile([D, 128], F32, tag="pt")
                nc.tensor.transpose(out=pt[:], in_=nm_sb[:], identity=ident[:])
                cat = sb.tile([2 * D, 128], F32, tag="cat")
                nc.sync.dma_start(out=cat[0:D, :], in_=xTc[:, tt * 128:(tt + 1) * 128])
                nc.any.tensor_copy(out=cat[D:2 * D, :], in_=pt[:])
                po = psum_t.tile([D, 128], F32, tag="po")
                nc.tensor.matmul(out=po[:], lhsT=wot_sb[:], rhs=cat[:], start=True, stop=True)
                if cfg["use_bo"]:
                    nc.scalar.activation(out=ostg[:, j * 128:(j + 1) * 128], in_=po[:],
                                         func=AF.Relu, bias=bo_sb[:], scale=1.0)
                else:
                    nc.scalar.activation(out=ostg[:, j * 128:(j + 1) * 128], in_=po[:],
                                         func=AF.Relu)
            nc.sync.dma_start(out=node_outT[:, g * 1024:(g + 1) * 1024], in_=ostg[:])


# ====================== host side ======================

def idx_layout(idx_flat):
    """[n] int -> [128, n/16] int16; index i at [i%16, i//16], replicated 8x."""
    n = idx_flat.shape[0]
    a = idx_flat.reshape(n // 16, 16).T.astype(np.int16)
    return np.tile(a, (8, 1)).copy()


def prep_inputs(x, edge_index, edge_attr, W_i, W_h, W_o, b_o, ln_gamma, ln_beta,
                cfg):
    """Returns (in_maps, meta) for run_bass_kernel_spmd."""
    NCORES = cfg["ncores"]
    N_PC = cfg["N_PC"]; N_TAB = cfg["N_TAB"]; N_LO = cfg["N_LO"]
    P_L, P_H = cfg["P_L"], cfg["P_H"]; P_E = P_L + P_H
    D = cfg["D"]; EDIM = cfg["EDIM"]
    NPC_REAL = cfg["NPC_REAL"]
    n_nodes = x.shape[0]
    E = edge_index.shape[1]

    src, dst = edge_index[0].astype(np.int64), edge_index[1].astype(np.int64)
    core_of = np.minimum(src * 0 + dst // NPC_REAL, NCORES - 1)
    new_of = lambda old: (np.minimum(old // NPC_REAL, NCORES - 1) * N_PC
                          + (old - np.minimum(old // NPC_REAL, NCORES - 1) * NPC_REAL))
    src_new = new_of(src)
    dst_new = new_of(dst)

    xT_np = np.zeros((D, N_TAB), np.float32)
    new_node_ids = new_of(np.arange(n_nodes))
    xT_np[:, new_node_ids] = np.asarray(x, np.float32).T

    w1t = np.ascontiguousarray(np.asarray(W_i)[:, :D].T, dtype=np.float32)
    w2t = np.ascontiguousarray(np.asarray(W_i)[:, D:].T, dtype=np.float32)
    wht = np.ascontiguousarray(np.asarray(W_h).T, dtype=np.float32)
    wot = np.ascontiguousarray(np.asarray(W_o).T, dtype=np.float32)
    gb = np.concatenate([np.tile(np.asarray(ln_gamma, np.float32)[None, :], (128, 1)),
                         np.tile(np.asarray(ln_beta, np.float32)[None, :], (128, 1))],
                        axis=1).astype(np.float32)
    boc = np.asarray(b_o, np.float32).reshape(D, 1)

    edge_attr = np.asarray(edge_attr, np.float32)
    SBLK = cfg["SBLK"]

    def assign_section(eids_sec, dst_loc_sec, n_calls):
        """Assign edges to scatter calls so no call repeats a dst.
        Returns stream positions (within section) for each edge."""
        order = np.argsort(dst_loc_sec, kind="stable")
        se, sd = eids_sec[order], dst_loc_sec[order]
        n = len(se)
        fill = np.zeros(n_calls, np.int64)
        pos = np.empty(n, np.int64)
        i = 0
        while i < n:
            j = i
            while j < n and sd[j] == sd[i]:
                j += 1
            deg = j - i
            assert deg <= n_calls, (deg, n_calls)
            cidx = int(sd[i]) % n_calls
            for k in range(i, j):
                probes = 0
                while fill[cidx] >= SBLK:
                    cidx = (cidx + 1) % n_calls
                    probes += 1
                    assert probes <= n_calls, "scatter call assignment overflow"
                pos[k] = cidx * SBLK + fill[cidx]
                fill[cidx] += 1
                cidx = (cidx + 1) % n_calls
            i = j
        return se, pos

    in_maps, meta = [], []
    for c in range(NCORES):
        eids = np.nonzero(core_of == c)[0]
        s_new = src_new[eids]
        d_loc = dst_new[eids] - c * N_PC
        isL = s_new < N_LO
        eL, pL = assign_section(eids[isL], d_loc[isL], P_L // SBLK)
        eH, pH = assign_section(eids[~isL], d_loc[~isL], P_H // SBLK)
        assert len(eL) <= P_L and len(eH) <= P_H, (len(eL), len(eH), P_L, P_H)

        pos_eids = np.full(P_E, -1, np.int64)
        pos_eids[pL] = eL
        pos_eids[P_L + pH] = eH

        gl = np.zeros(P_L, np.int64)
        gl[pL] = src_new[eL]
        gh = np.zeros(P_H, np.int64)
        gh[pH] = src_new[eH] - N_LO
        # default: per-slot-unique trash rows (slot j in any call -> N_PC + j)
        sl = np.tile(np.arange(SBLK, dtype=np.int64), P_E // SBLK) + N_PC
        sl[pL] = dst_new[eL] - c * N_PC
        sl[P_L + pH] = dst_new[eH] - c * N_PC
        # verify uniqueness within every scatter call
        for cb in range(P_E // SBLK):
            blk = sl[cb * SBLK:(cb + 1) * SBLK]
            assert len(np.unique(blk)) == SBLK, f"dup idx in scatter call {cb}"

        at = np.zeros((EDIM, P_E), np.float32)
        valid = pos_eids >= 0
        at[:, valid] = edge_attr[pos_eids[valid]].T

        in_maps.append({
            "xT": xT_np,
            "xTc": np.ascontiguousarray(xT_np[:, c * N_PC:(c + 1) * N_PC]),
            "attrT": at,
            "gidx_lo": idx_layout(gl),
            "gidx_hi": idx_layout(gh),
            "sidx": idx_layout(sl),
            "w1t": w1t, "w2t": w2t, "wht": wht, "wot": wot,
            "gb": gb, "boc": boc,
        })
        meta.append(pos_eids)
    return in_maps, meta


def assemble_outputs(results, meta, cfg, n_nodes, n_edges):
    NCORES = cfg["ncores"]
    N_PC = cfg["N_PC"]; NPC_REAL = cfg["NPC_REAL"]; D = cfg["D"]
    nm_full = np.zeros((n_edges, D), np.float32)
    node_out = np.zeros((n_nodes, D), np.float32)
    for c in range(NCORES):
        nm = results[c]["nm_out"].transpose(1, 0, 2).reshape(-1, D)
        pos_eids = meta[c]
        valid = pos_eids >= 0
        nm_full[pos_eids[valid]] = nm[valid]
        no = results[c]["node_outT"].T
        lo = c * NPC_REAL
        hi = min(n_nodes, (c + 1) * NPC_REAL) if c < NCORES - 1 else n_nodes
        node_out[lo:hi] = no[:hi - lo]
    return node_out, nm_full


def make_cfg(n_nodes, edge_index, ncores=8, gblk=8192, sblk=2048,
             use_gamma_beta=True, use_bo=True):
    NPC_REAL = -(-n_nodes // ncores)  # ceil
    # padded per-core rows: multiple of 1024 >= NPC_REAL
    N_PC = ((NPC_REAL + 1023) // 1024) * 1024
    N_TAB = N_PC * ncores
    N_LO = N_TAB // 2
    src, dst = edge_index[0].astype(np.int64), edge_index[1].astype(np.int64)
    core_of = np.minimum(dst // NPC_REAL, ncores - 1)
    c_new = np.minimum(src // NPC_REAL, ncores - 1)
    src_new = c_new * N_PC + (src - c_new * NPC_REAL)
    maxL = maxH = 1
    maxdegL = maxdegH = 1
    for c in range(ncores):
        m = core_of == c
        s = src_new[m]
        d = dst[m]
        isL = s < N_LO
        maxL = max(maxL, int(isL.sum()))
        maxH = max(maxH, int((~isL).sum()))
        if isL.sum():
            maxdegL = max(maxdegL, int(np.bincount(d[isL]).max()))
        if (~isL).sum():
            maxdegH = max(maxdegH, int(np.bincount(d[~isL]).max()))
    def psize(cnt, maxdeg):
        # calls must cover count (with ~6% slack) and exceed max dst degree
        ncalls = max(-(-int(cnt * 1.06) // sblk), maxdeg + 2)
        p = ncalls * sblk
        return -(-p // gblk) * gblk  # round up to gather-block multiple
    P_L = psize(maxL, maxdegL)
    P_H = psize(maxH, maxdegH)
    return dict(
        D=64, EDIM=16, GBLK=gblk, SBLK=sblk, P_L=P_L, P_H=P_H,
        N_TAB=N_TAB, N_LO=N_LO, N_PC=N_PC,
        N_AGGROWS=N_PC + sblk, NPC_REAL=NPC_REAL,
        LN_EPS=1e-5, ncores=ncores,
        use_gamma_beta=use_gamma_beta, use_bo=use_bo)


# ====================== public entry point ======================

_TRACE = [False]
_LAST_RESULT = [None]


def kernel(x, edge_index, edge_attr, W_i, W_h, W_o, b_o, ln_gamma, ln_beta):
    """Full-input entry point: shards across 8 NeuronCores internally."""
    x = np.asarray(x, np.float32)
    edge_index = np.asarray(edge_index)
    edge_attr = np.asarray(edge_attr, np.float32)
    n_nodes, n_edges = x.shape[0], edge_index.shape[1]

    use_gb = not (np.allclose(np.asarray(ln_gamma), 1.0)
                  and np.allclose(np.asarray(ln_beta), 0.0))
    use_bo = not np.allclose(np.asarray(b_o), 0.0)
    cfg = make_cfg(n_nodes, edge_index, gblk=2048, sblk=2048,
                   use_gamma_beta=use_gb, use_bo=use_bo)
    nc = build_graph(cfg)
    in_maps, meta = prep_inputs(x, edge_index, edge_attr, W_i, W_h, W_o,
                                b_o, ln_gamma, ln_beta, cfg)
    res = bass_utils.run_bass_kernel_spmd(
        nc, in_maps, core_ids=list(range(cfg["ncores"])), trace=_TRACE[0])
    _LAST_RESULT[0] = res
    node_out, nm_full = assemble_outputs(res.results, meta, cfg,
                                         n_nodes, n_edges)
    return node_out, nm_full
